# revision 29
# baseline (speedup 1.0000x reference)
"""Trainium2 Bass kernel for gated 1x1-conv attention (dense_transformer).

Problem structure (B=4, C=3, H=W=64, heads=3 => c_h=1): attention logits are
rank-1: att[n] = softmax_m(q_n * k_m) @ v, N=4096 pixels. A luma gate scales
q; the 1x1 convs are 3x3 channel mixes.

Sharding: 8 cores = (batch b = j//2) x (query-pixel half = j%2); each core
produces the full RGB output for its 2048 query pixels. No collectives.

v2 algorithm (Gaussian-quadrature factorization of the exp kernel):
  exp(q k) = e^{-s^2 k^2/2} * (h/(s sqrt(2pi))) * sum_j e^{-(q-t_j)^2/(2s^2)} e^{t_j k}
over a T=128 grid t_j. This collapses the N x N attention into N x T + T x N
work (constants cancel in the softmax ratio):
  grid:  gnum[j] = sum_m (c_m v_m) e^{t_j k_m},  gden[j] = sum_m c_m e^{t_j k_m}
         with c_m = e^{-s^2 k_m^2 / 2}   (ScalarE exp + TensorE matmul)
  rbf:   W[j, n] = e^{-(q_n - t_j)^2/(2 s^2)}    (ScalarE square+exp)
         att[n] = (W.T @ gnum) / (W.T @ gden)    (TensorE)
Max rel err vs exact softmax ~4e-3 (bf16 matmuls), verified in numpy.
"""

import numpy as np

import concourse.bass as bass
import concourse.bacc as bacc
import concourse.mybir as mybir
from concourse.tile import TileContext
from concourse.bass_utils import run_bass_kernel_spmd

F32 = mybir.dt.float32
BF16 = mybir.dt.bfloat16
AF = mybir.ActivationFunctionType
ALU = mybir.AluOpType

N = 4096          # pixels per image
NSL = 2048        # query pixels per core
NMT = 32          # key (m) tiles of 128
NQT = 16          # query tiles of 128
P = 128
T = 128           # Gaussian-quadrature grid size
T0, T1 = -2.6, 1.7
HG = (T1 - T0) / (T - 1)
SIG = 1.25 * HG
ISQ = 1.0 / (SIG * np.sqrt(2.0))   # 1/(sigma*sqrt(2))
LUMW = (0.299, 0.587, 0.114)


def build_nc(debug=False):
    nc = bacc.Bacc("TRN2", target_bir_lowering=False, debug=False,
                   num_devices=8)

    img = nc.declare_dram_parameter("img", [3, N], F32, isOutput=False)
    qimg = nc.declare_dram_parameter("qimg", [3, NSL], F32, isOutput=False)
    qimgT = nc.declare_dram_parameter("qimgT", [P, 3 * NQT], F32, isOutput=False)
    wkvl = nc.declare_dram_parameter("wkvl", [3, 8], F32, isOutput=False)
    wql = nc.declare_dram_parameter("wql", [3, 4], F32, isOutput=False)
    wocol = nc.declare_dram_parameter("wocol", [P, 9], F32, isOutput=False)
    tbc = nc.declare_dram_parameter("tbc", [P, T], F32, isOutput=False)
    tsig = nc.declare_dram_parameter("tsig", [P, 1], F32, isOutput=False)
    sigk = nc.declare_dram_parameter("sigk", [P, 1], F32, isOutput=False)
    tcol = nc.declare_dram_parameter("tcol", [P, 1], F32, isOutput=False)
    isq = nc.declare_dram_parameter("isq", [P, 1], F32, isOutput=False)
    out = nc.declare_dram_parameter("out", [P, 3 * NQT], F32, isOutput=True)
    if debug:
        dbg_g = nc.declare_dram_parameter("dbg_g", [P, 6], F32, isOutput=True)
        dbg_att = nc.declare_dram_parameter("dbg_att", [P, 3 * NQT], F32,
                                            isOutput=True)
        dbg_qp = nc.declare_dram_parameter("dbg_qp", [P, 3 * NQT], F32,
                                           isOutput=True)
        dbg_nd = nc.declare_dram_parameter("dbg_nd", [P, 3 * 2 * NQT], F32,
                                           isOutput=True)
        dbg_w = nc.declare_dram_parameter("dbg_w", [P, NSL], F32,
                                          isOutput=True)

    with TileContext(nc) as tc:
        with (
            tc.tile_pool(name="singles", bufs=1) as singles,
            tc.tile_pool(name="sbuf", bufs=2) as sb,
            tc.tile_pool(name="stile", bufs=3) as stile,
            tc.tile_pool(name="wtile", bufs=2) as wtile,
            tc.tile_pool(name="psum", bufs=1, space="PSUM") as ps,
            tc.tile_pool(name="psum_g", bufs=1, space="PSUM") as psg,
            tc.tile_pool(name="psum_att", bufs=2, space="PSUM") as psa,
            tc.tile_pool(name="psum_qb", bufs=2, space="PSUM") as psq,
            tc.tile_pool(name="dram", bufs=1, space="DRAM") as dpool,
        ):
            # ---- load inputs ----
            img_sb = singles.tile([3, N], F32)
            nc.sync.dma_start(out=img_sb[:], in_=img[:])
            qimg_sb = singles.tile([3, NSL], F32)
            nc.sync.dma_start(out=qimg_sb[:], in_=qimg[:])
            qimgT_sb = singles.tile([P, 3 * NQT], F32)
            nc.sync.dma_start(out=qimgT_sb[:], in_=qimgT[:])
            wkvl_sb = singles.tile([3, 8], F32)
            nc.sync.dma_start(out=wkvl_sb[:], in_=wkvl[:])
            wql_sb = singles.tile([3, 4], F32)
            nc.sync.dma_start(out=wql_sb[:], in_=wql[:])
            wocol_sb = singles.tile([P, 9], F32)
            nc.sync.dma_start(out=wocol_sb[:], in_=wocol[:])
            tbc_sb = singles.tile([P, T], F32)
            nc.sync.dma_start(out=tbc_sb[:], in_=tbc[:])
            tsig_sb = singles.tile([P, 1], F32)
            nc.sync.dma_start(out=tsig_sb[:], in_=tsig[:])
            sigk_sb = singles.tile([P, 1], F32)
            nc.sync.dma_start(out=sigk_sb[:], in_=sigk[:])
            tcol_sb = singles.tile([P, 1], F32)
            nc.sync.dma_start(out=tcol_sb[:], in_=tcol[:])
            isq_sb = singles.tile([P, 1], F32)
            nc.sync.dma_start(out=isq_sb[:], in_=isq[:])

            ones_1x128 = singles.tile([1, P], F32)
            nc.vector.memset(ones_1x128[:], 1.0)
            ones_bf = singles.tile([1, P], BF16)
            nc.vector.memset(ones_bf[:], 1.0)
            ones_sq = singles.tile([P, P], F32)
            nc.vector.memset(ones_sq[:], 1.0)
            WARMUP = True
            if WARMUP:
                warm_bf = singles.tile([P, 512], BF16)
                nc.vector.memset(warm_bf[:], 1.0)
                warm_ps = psg.tile([P, 512], F32, tag="g")
                for _ in range(11):
                    nc.tensor.matmul(warm_ps[:], lhsT=warm_bf[:, 0:P],
                                     rhs=warm_bf[:], start=True, stop=True)

            # ---- conv pass over keys: k, v, L columns ----
            psum_conv = ps.tile([P, NMT, 8], F32, tag="big")
            for mt in range(NMT):
                nc.tensor.matmul(
                    psum_conv[:, mt, :],
                    lhsT=img_sb[:, mt * P:(mt + 1) * P],
                    rhs=wkvl_sb[:],
                    start=True, stop=True,
                )
            k_sb = singles.tile([P, NMT, 3], F32)
            nc.vector.tensor_copy(k_sb[:], psum_conv[:, :, 0:3])
            L_sb = singles.tile([P, NMT], F32)
            nc.vector.tensor_copy(L_sb[:], psum_conv[:, :, 6])

            # c = exp(-(k*sig)^2/2); weight cv = c*v (bf16 is enough for a
            # weight); k splits into bf16 hi+lo so the broadcast matmuls keep
            # ~2e-5 relative precision on the exponent input.
            csq = sb.tile([P, NMT, 3], F32)
            nc.scalar.activation(csq[:], k_sb[:], AF.Square,
                                 scale=sigk_sb[:, 0:1])
            c_sb = sb.tile([P, NMT, 3], F32)
            nc.scalar.activation(c_sb[:], csq[:], AF.Exp, scale=-1.0)
            cv_cols = sb.tile([P, NMT, 3], BF16)
            nc.vector.tensor_tensor(cv_cols[:], psum_conv[:, :, 3:6],
                                    c_sb[:], op=ALU.mult)
            khi_cols = sb.tile([P, NMT, 3], BF16)
            nc.vector.tensor_copy(khi_cols[:], k_sb[:])
            klo_f = sb.tile([P, NMT, 3], F32)
            nc.vector.tensor_tensor(klo_f[:], k_sb[:], khi_cols[:],
                                    op=ALU.subtract)
            klo_cols = sb.tile([P, NMT, 3], BF16)
            nc.vector.tensor_copy(klo_cols[:], klo_f[:])
            # columns -> DRAM rows (transposing DMAs)
            krow_d = dpool.tile([3, 3, N], BF16)   # [hi|lo|cv][h][m]
            for i, cols in enumerate((khi_cols, klo_cols, cv_cols)):
                for hh in range(3):
                    view = bass.AP(tensor=krow_d.tensor,
                                   offset=krow_d.offset + i * 3 * N + hh * N,
                                   ap=[[1, P], [P, NMT]])
                    nc.sync.dma_start(out=view, in_=cols[:, :, hh])

            # ---- conv pass over queries: q cols + Lq col ----
            psum_q = ps.tile([P, NQT, 4], F32, tag="big")
            for qt in range(NQT):
                nc.tensor.matmul(
                    psum_q[:, qt, :],
                    lhsT=qimg_sb[:, qt * P:(qt + 1) * P],
                    rhs=wql_sb[:],
                    start=True, stop=True,
                )

            # ---- luma stats (replicated to all 128 partitions) ----
            Lr = sb.tile([P, 1], F32)
            nc.vector.tensor_reduce(Lr[:], L_sb[:], axis=mybir.AxisListType.X,
                                    op=ALU.add)
            mu_ps = psg.tile([P, 1], F32, tag="g")
            nc.tensor.matmul(mu_ps[:], lhsT=ones_sq[:], rhs=Lr[:],
                             start=True, stop=True)
            mu_sb = sb.tile([P, 1], F32)
            nc.vector.tensor_scalar_mul(mu_sb[:], mu_ps[:], 1.0 / N)

            dltmp = sb.tile([P, NMT], F32)
            nc.vector.tensor_scalar(dltmp[:], L_sb[:], mu_sb[:, 0:1], None,
                                    op0=ALU.subtract)
            sr = sb.tile([P, 2], F32)
            nc.vector.tensor_reduce(sr[:, 0:1], dltmp[:],
                                    axis=mybir.AxisListType.X,
                                    op=ALU.add, apply_absolute_value=True)
            dl2 = sb.tile([P, NMT], F32)
            nc.vector.tensor_tensor(dl2[:], dltmp[:], dltmp[:], op=ALU.mult)
            nc.vector.tensor_reduce(sr[:, 1:2], dl2[:],
                                    axis=mybir.AxisListType.X, op=ALU.add)
            stats_ps = psg.tile([P, 2], F32, tag="g")
            nc.tensor.matmul(stats_ps[:], lhsT=ones_sq[:], rhs=sr[:],
                             start=True, stop=True)
            stats_sb = sb.tile([P, 2], F32)
            nc.vector.tensor_copy(stats_sb[:], stats_ps[:])
            s1sq = sb.tile([P, 1], F32)
            nc.vector.tensor_tensor(s1sq[:], stats_sb[:, 0:1],
                                    stats_sb[:, 0:1], op=ALU.mult)
            var_sb = sb.tile([P, 1], F32)
            nc.vector.scalar_tensor_tensor(var_sb[:], in0=s1sq[:],
                                           scalar=-1.0 / N,
                                           in1=stats_sb[:, 1:2],
                                           op0=ALU.mult, op1=ALU.add)
            nc.vector.tensor_scalar_mul(var_sb[:], var_sb[:], 1.0 / (N - 1))
            # std = sqrt(var) + 1e-6 via exp(0.5 ln var); rneg = -1/std
            lnv = sb.tile([P, 1], F32)
            nc.scalar.activation(lnv[:], var_sb[:], AF.Ln)
            stdv = sb.tile([P, 1], F32)
            nc.scalar.activation(stdv[:], lnv[:], AF.Exp, scale=0.5)
            nc.vector.tensor_scalar_add(stdv[:], stdv[:], 1e-6)
            rneg = sb.tile([P, 1], F32)
            nc.vector.reciprocal(rneg[:], stdv[:])
            nc.vector.tensor_scalar_mul(rneg[:], rneg[:], -1.0)

            # ---- gate in column layout; q' = q * (1 + sigmoid(dL/std)) ----
            dlqc = sb.tile([P, NQT], F32)
            nc.vector.tensor_scalar(dlqc[:], psum_q[:, :, 3], mu_sb[:, 0:1],
                                    None, op0=ALU.subtract)
            nc.vector.scalar_tensor_tensor(dlqc[:], in0=dlqc[:], scalar=-1.0,
                                           in1=dlqc[:], op0=ALU.mult,
                                           op1=ALU.max)
            eg = sb.tile([P, NQT], F32)
            nc.scalar.activation(eg[:], dlqc[:], AF.Exp, scale=rneg[:])
            nc.vector.tensor_scalar_add(eg[:], eg[:], 1.0)
            opg = sb.tile([P, NQT], F32)
            nc.vector.reciprocal(opg[:], eg[:])
            nc.vector.tensor_scalar_add(opg[:], opg[:], 1.0)  # 1 + gate

            qp_cols = sb.tile([P, 3, NQT], F32)
            for h in range(3):
                nc.vector.tensor_tensor(qp_cols[:, h, :], psum_q[:, :, h],
                                        opg[:], op=ALU.mult)
            if debug:
                nc.sync.dma_start(out=dbg_qp[:],
                                  in_=qp_cols[:].rearrange("p h q -> p (h q)"))

            # q' hi/lo bf16 split, columns -> DRAM rows
            qhi_cols = sb.tile([P, 3, NQT], BF16)
            nc.vector.tensor_copy(qhi_cols[:], qp_cols[:])
            qlo_f = sb.tile([P, 3, NQT], F32)
            nc.vector.tensor_tensor(qlo_f[:], qp_cols[:], qhi_cols[:],
                                    op=ALU.subtract)
            qlo_cols = sb.tile([P, 3, NQT], BF16)
            nc.vector.tensor_copy(qlo_cols[:], qlo_f[:])
            qrow_d = dpool.tile([2, 3, NSL], BF16)
            for i, cols in enumerate((qhi_cols, qlo_cols)):
                for hh in range(3):
                    view = bass.AP(tensor=qrow_d.tensor,
                                   offset=qrow_d.offset + i * 3 * NSL + hh * NSL,
                                   ap=[[1, P], [P, NQT]])
                    nc.sync.dma_start(out=view, in_=cols[:, hh, :])

            # ---- per-head: grid build (t on partitions) + RBF ----
            NCH = 4           # m chunks of 1024 for the broadcast pipeline
            CH = N // NCH
            att_sb = singles.tile([P, 3, NQT], F32)
            for h in range(3):
                # per-head row slices at partition 0
                khi_r = sb.tile([1, N], BF16, tag="khr")
                nc.sync.dma_start(out=khi_r[:], in_=krow_d[0:1, h, :])
                klo_r = sb.tile([1, N], BF16, tag="klr")
                nc.sync.dma_start(out=klo_r[:], in_=krow_d[1:2, h, :])
                cv_r = sb.tile([1, N], BF16, tag="cvr")
                nc.sync.dma_start(out=cv_r[:], in_=krow_d[2:3, h, :])

                dpart = sb.tile([P, NCH], F32, tag="dpart")
                npart = sb.tile([P, NCH], F32, tag="npart")
                for ch in range(NCH):
                    # k broadcast (hi+lo accumulate) into PSUM
                    kb_ps = psq.tile([P, CH], F32, tag="bc")
                    for half in range(2):
                        nc.tensor.matmul(
                            kb_ps[:, half * 512:(half + 1) * 512],
                            lhsT=ones_bf[:],
                            rhs=khi_r[0:1, ch * CH + half * 512:
                                      ch * CH + (half + 1) * 512],
                            start=True, stop=False)
                        nc.tensor.matmul(
                            kb_ps[:, half * 512:(half + 1) * 512],
                            lhsT=ones_bf[:],
                            rhs=klo_r[0:1, ch * CH + half * 512:
                                      ch * CH + (half + 1) * 512],
                            start=False, stop=True)
                    # S = exp(t_j * k_m), den partial via accum_out
                    s_t = stile.tile([P, CH], BF16, tag="s")
                    USE_ACC = True
                    if USE_ACC:
                        nc.scalar.activation(s_t[:], kb_ps[:], AF.Exp,
                                             scale=tcol_sb[:, 0:1],
                                             accum_out=dpart[:, ch:ch + 1])
                    else:
                        nc.scalar.activation(s_t[:], kb_ps[:], AF.Exp,
                                             scale=tcol_sb[:, 0:1])
                        nc.vector.tensor_reduce(dpart[:, ch:ch + 1], s_t[:],
                                                axis=mybir.AxisListType.X,
                                                op=ALU.add)
                    # cv broadcast, then num partial via fused mult-reduce
                    cv_ps = psq.tile([P, CH], F32, tag="bc")
                    for half in range(2):
                        nc.tensor.matmul(
                            cv_ps[:, half * 512:(half + 1) * 512],
                            lhsT=ones_bf[:],
                            rhs=cv_r[0:1, ch * CH + half * 512:
                                     ch * CH + (half + 1) * 512],
                            start=True, stop=True)
                    junk = stile.tile([P, CH], BF16, tag="junk")
                    USE_TTR = False
                    if USE_TTR:
                        nc.vector.tensor_tensor_reduce(
                            out=junk[:], in0=s_t[:], in1=cv_ps[:], scale=1.0,
                            scalar=0.0, op0=ALU.mult, op1=ALU.add,
                            accum_out=npart[:, ch:ch + 1])
                    else:
                        nc.vector.tensor_tensor(junk[:], s_t[:], cv_ps[:],
                                                op=ALU.mult)
                        nc.vector.tensor_reduce(npart[:, ch:ch + 1], junk[:],
                                                axis=mybir.AxisListType.X,
                                                op=ALU.add)
                g2f = sb.tile([P, 2], F32)
                nc.vector.tensor_reduce(g2f[:, 0:1], npart[:],
                                        axis=mybir.AxisListType.X, op=ALU.add)
                nc.vector.tensor_reduce(g2f[:, 1:2], dpart[:],
                                        axis=mybir.AxisListType.X, op=ALU.add)
                g2_sb = sb.tile([P, 2], BF16)
                nc.vector.tensor_copy(g2_sb[:], g2f[:])
                if debug:
                    nc.sync.dma_start(out=dbg_g[:, 2 * h:2 * h + 2],
                                      in_=g2f[:])

                # q' broadcast (hi+lo) and W = exp(-((q - t_j) isq)^2)
                qhi_r = sb.tile([1, NSL], BF16, tag="qhr")
                nc.sync.dma_start(out=qhi_r[:], in_=qrow_d[0:1, h, :])
                qlo_r = sb.tile([1, NSL], BF16, tag="qlr")
                nc.sync.dma_start(out=qlo_r[:], in_=qrow_d[1:2, h, :])
                z2 = wtile.tile([P, NSL], F32, tag="z2")
                for ch in range(2):
                    qb_ps = psq.tile([P, 1024], F32, tag="bc")
                    for half in range(2):
                        off = ch * 1024 + half * 512
                        nc.tensor.matmul(
                            qb_ps[:, half * 512:(half + 1) * 512],
                            lhsT=ones_bf[:],
                            rhs=qhi_r[0:1, off:off + 512],
                            start=True, stop=False)
                        nc.tensor.matmul(
                            qb_ps[:, half * 512:(half + 1) * 512],
                            lhsT=ones_bf[:],
                            rhs=qlo_r[0:1, off:off + 512],
                            start=False, stop=True)
                    nc.scalar.activation(z2[:, ch * 1024:(ch + 1) * 1024],
                                         qb_ps[:], AF.Square,
                                         scale=isq_sb[:, 0:1],
                                         bias=tsig_sb[:, 0:1])
                w_sb = wtile.tile([P, NSL], BF16, tag="w")
                nc.scalar.activation(w_sb[:], z2[:], AF.Exp, scale=-1.0)

                # att columns: [n_chunk, (num, den)] per query tile
                psum_att = psa.tile([P, 2 * NQT], F32, tag="att")
                for qt in range(NQT):
                    nc.tensor.matmul(psum_att[:, 2 * qt:2 * qt + 2],
                                     lhsT=w_sb[:, qt * P:(qt + 1) * P],
                                     rhs=g2_sb[:],
                                     start=True, stop=True)
                rden = sb.tile([P, NQT], F32)
                nc.vector.reciprocal(
                    rden[:],
                    psum_att[:].rearrange("p (q two) -> p q two", two=2)[:, :, 1])
                nc.vector.tensor_tensor(
                    att_sb[:, h, :],
                    psum_att[:].rearrange("p (q two) -> p q two", two=2)[:, :, 0],
                    rden[:], op=ALU.mult)

            if debug:
                nc.sync.dma_start(out=dbg_att[:],
                                  in_=att_sb[:].rearrange("p h q -> p (h q)"))

            # ---- output mix (wo), residual, clip -- all in column layout ----
            out_sb = singles.tile([P, 3, NQT], F32)
            for cch in range(3):
                mix = sb.tile([P, NQT], F32, tag="mix")
                nc.vector.tensor_scalar(mix[:], att_sb[:, 0, :],
                                        wocol_sb[:, 3 * cch:3 * cch + 1], None,
                                        op0=ALU.mult)
                for h in (1, 2):
                    nc.vector.scalar_tensor_tensor(
                        mix[:], in0=att_sb[:, h, :],
                        scalar=wocol_sb[:, 3 * cch + h:3 * cch + h + 1],
                        in1=mix[:], op0=ALU.mult, op1=ALU.add)
                nc.vector.tensor_tensor(
                    mix[:], mix[:],
                    qimgT_sb[:, cch * NQT:(cch + 1) * NQT], op=ALU.add)
                nc.vector.tensor_scalar_max(mix[:], mix[:], 0.0)
                nc.vector.tensor_scalar_min(out_sb[:, cch, :], mix[:], 1.0)
            nc.sync.dma_start(out=out[:],
                              in_=out_sb[:].rearrange("p c q -> p (c q)"))

    nc.finalize()
    return nc


_NC_CACHE = {}


def _get_nc(debug=False):
    key = ("dbg" if debug else "nc")
    if key not in _NC_CACHE:
        _NC_CACHE[key] = build_nc(debug)
    return _NC_CACHE[key]


def make_in_maps(rgb, wq, wk, wv, wo):
    x = np.ascontiguousarray(rgb.reshape(4, 3, N)).astype(np.float32)
    lumw = np.array(LUMW, dtype=np.float32)
    wkvl = np.concatenate(
        [wk.T, wv.T, lumw[:, None], np.zeros((3, 1), np.float32)], axis=1
    ).astype(np.float32)
    wql = np.concatenate([wq.T, lumw[:, None]], axis=1).astype(np.float32)
    wocol = np.tile(wo.reshape(1, 9), (P, 1)).astype(np.float32)
    # runtime grid: |q'| <= 2 * max_h sum_c |wq[h,c]| since rgb in [0,1] and
    # (1+gate) <= 2; margin covers the 6-sigma Gaussian quadrature tails
    R = 2.0 * float(np.abs(wq).sum(axis=1).max()) + 1.0
    t0, t1 = -R, R
    hg = (t1 - t0) / (T - 1)
    sig = 1.25 * hg
    isqv = 1.0 / (sig * np.sqrt(2.0))
    tg = (t0 + np.arange(T) * hg).astype(np.float32)
    tbc = np.tile(tg[None, :], (P, 1)).astype(np.float32)
    tsig = (-tg * isqv).reshape(P, 1).astype(np.float32)
    sigk = np.full((P, 1), sig / np.sqrt(2.0), np.float32)
    isq = np.full((P, 1), isqv, np.float32)
    tcolv = tg.reshape(P, 1).astype(np.float32)

    in_maps = []
    for j in range(8):
        b, half = j // 2, j % 2
        sl = slice(half * NSL, (half + 1) * NSL)
        qs = x[b][:, sl]                         # [3, 2048]
        # qimgT[p, c*16+qt] = qs[c, qt*128+p]
        qT = np.ascontiguousarray(
            qs.reshape(3, NQT, P).transpose(2, 0, 1).reshape(P, 3 * NQT))
        in_maps.append({
            "img": x[b],
            "qimg": np.ascontiguousarray(qs),
            "qimgT": qT.astype(np.float32),
            "wkvl": wkvl,
            "wql": wql,
            "wocol": wocol,
            "tbc": tbc,
            "tsig": tsig,
            "sigk": sigk,
            "isq": isq,
            "tcol": tcolv,
        })
    return in_maps


def run(rgb, wq, wk, wv, wo, trace=False, debug=False):
    nc = _get_nc(debug)
    in_maps = make_in_maps(rgb, wq, wk, wv, wo)
    res = run_bass_kernel_spmd(nc, in_maps, core_ids=list(range(8)),
                               trace=trace)
    y = np.zeros((4, 3, N), dtype=np.float32)
    for j in range(8):
        b, half = j // 2, j % 2
        sl = slice(half * NSL, (half + 1) * NSL)
        o = res.results[j]["out"]                # [128, 3*16]
        y[b][:, sl] = o.reshape(P, 3, NQT).transpose(1, 2, 0).reshape(3, NSL)
    return y.reshape(4, 3, 64, 64), res


def kernel(**inputs):
    y, _ = run(inputs["rgb"], inputs["wq"], inputs["wk"], inputs["wv"],
               inputs["wo"])
    return y
